# revision 12
# baseline (speedup 1.0000x reference)
"""Self-contained 8-core Trainium2 Bass kernel for nn_MultiHeadAttention.

Full (unsharded) inputs in, full output out. Sharding: core c handles
batch b = c // 2, query-half h = c % 2 (1024 queries). K/V projections for a
batch are computed redundantly on the 2 cores sharing it -> zero collectives,
disjoint outputs.

All matmuls run in float32r (TF32-like, ~1.6e-4 rel err, 4x fp32 throughput).
"""

import numpy as np

import concourse.bass as bass
import concourse.mybir as mybir
from concourse import bacc
from concourse.tile import TileContext
from concourse.bass_utils import run_bass_kernel_spmd

F32 = mybir.dt.float32
F32R = mybir.dt.float32r
ACT = mybir.ActivationFunctionType

B, S, D = 4, 2048, 1024
H, DK = 16, 64
SQ = S // 2            # queries per core
P = 128
NCORES = 8
FC = D // P            # 8 feature chunks (contraction)
OFC = D // P           # 8 output-feature chunks
PAIRS = H // 2         # 8 head pairs (2 heads = 128 partitions)
NKT = S // P           # 16 key tiles of 128 tokens
QTW = 512              # q tile width
NQT = SQ // QTW        # 2
SCALE = 1.0 / np.sqrt(np.float32(DK))


def build_nc():
    nc = bacc.Bacc()

    xq = nc.declare_dram_parameter("xq_t", [D, SQ], F32R, isOutput=False)
    xk = nc.declare_dram_parameter("xk_t", [D, S], F32R, isOutput=False)
    xv = nc.declare_dram_parameter("xv_t", [D, S], F32R, isOutput=False)
    wq = nc.declare_dram_parameter("wq_t", [D, D], F32R, isOutput=False)
    wk = nc.declare_dram_parameter("wk_t", [D, D], F32R, isOutput=False)
    wv = nc.declare_dram_parameter("wv_t", [D, D], F32R, isOutput=False)
    wo = nc.declare_dram_parameter("wo_t", [D, D], F32R, isOutput=False)
    bq = nc.declare_dram_parameter("b_q_r", [P, OFC], F32, isOutput=False)
    bk = nc.declare_dram_parameter("b_k_r", [P, OFC], F32, isOutput=False)
    bo = nc.declare_dram_parameter("b_o_r", [P, OFC], F32, isOutput=False)
    ones_in = nc.declare_dram_parameter("ones_row", [1, P], F32, isOutput=False)
    vones_in = nc.declare_dram_parameter("v_ones", [P, NKT, 2, 1], F32R,
                                         isOutput=False)
    out = nc.declare_dram_parameter("out_t", [D, SQ], F32, isOutput=True)

    kt_scr = nc.dram_tensor("kt_scr", [P, PAIRS, S], F32R)
    v_scr = nc.dram_tensor("v_scr", [P, PAIRS, NKT, 2, DK], F32R)

    with nc.allow_low_precision(reason="f32r compute"), TileContext(nc) as tc:
        with (
            tc.tile_pool(name="persist", bufs=1) as pers,
            tc.tile_pool(name="norm", bufs=2) as npool,
        ):
            qt_s = pers.tile([P, OFC, SQ], F32R, tag="qt")
            attn_t = pers.tile([P, PAIRS, SQ], F32R, tag="attnt")
            tbq = pers.tile([P, OFC], F32, tag="tbq")
            tbk = pers.tile([P, OFC], F32, tag="tbk")
            tbo = pers.tile([P, OFC], F32, tag="tbo")
            tones = pers.tile([1, P], F32, tag="tones")
            vones_s = pers.tile([P, NKT, 2, 1], F32R, tag="vones")
            nc.sync.dma_start(tbq[:], bq[:])
            nc.sync.dma_start(tbk[:], bk[:])
            nc.sync.dma_start(tbo[:], bo[:])
            nc.sync.dma_start(tones[:], ones_in[:])
            nc.sync.dma_start(vones_s[:], vones_in[:])

            # ---------------- Stage A: projections ----------------
            with (
                tc.tile_pool(name="wpool", bufs=2) as wpool,
                tc.tile_pool(name="xpool", bufs=2) as xpool,
                tc.tile_pool(name="kspool", bufs=3) as kspool,
                tc.tile_pool(name="apsum", bufs=4, space="PSUM") as apsum,
            ):
                # --- Q projection: QT[of, t] = Wq @ xq ---
                for half in range(2):  # of-halves of 512
                    wtile = wpool.tile([P, FC, D // 2], F32R, tag="w_half")
                    nc.sync.dma_start(
                        wtile[:],
                        wq[:, half * 512:(half + 1) * 512].rearrange(
                            "(fc p) o -> p fc o", p=P),
                    )
                    for tt in range(NQT):
                        xts = []
                        for fc in range(FC):
                            xt = xpool.tile([P, QTW], F32R, tag=f"x{fc}")
                            nc.sync.dma_start(
                                xt[:], xq[fc * P:(fc + 1) * P,
                                          tt * QTW:(tt + 1) * QTW])
                            xts.append(xt)
                        for oi in range(4):
                            ofc = half * 4 + oi
                            ps = apsum.tile([P, QTW], F32, tag="acc")
                            for fc in range(FC):
                                nc.tensor.matmul(
                                    ps[:],
                                    wtile[:, fc, oi * P:(oi + 1) * P],
                                    xts[fc][:],
                                    start=(fc == 0), stop=(fc == FC - 1),
                                )
                            nc.scalar.activation(
                                qt_s[:, ofc, tt * QTW:(tt + 1) * QTW], ps[:],
                                ACT.Identity, bias=tbq[:, ofc:ofc + 1])

                # --- K projection -> kt_scr[of, t] (DRAM bounce) ---
                for half in range(2):
                    wtile = wpool.tile([P, FC, D // 2], F32R, tag="w_half")
                    nc.sync.dma_start(
                        wtile[:],
                        wk[:, half * 512:(half + 1) * 512].rearrange(
                            "(fc p) o -> p fc o", p=P),
                    )
                    for tt in range(S // QTW):
                        xts = []
                        for fc in range(FC):
                            xt = xpool.tile([P, QTW], F32R, tag=f"x{fc}")
                            nc.sync.dma_start(
                                xt[:], xk[fc * P:(fc + 1) * P,
                                          tt * QTW:(tt + 1) * QTW])
                            xts.append(xt)
                        for oi in range(4):
                            ofc = half * 4 + oi
                            ps = apsum.tile([P, QTW], F32, tag="acc")
                            for fc in range(FC):
                                nc.tensor.matmul(
                                    ps[:],
                                    wtile[:, fc, oi * P:(oi + 1) * P],
                                    xts[fc][:],
                                    start=(fc == 0), stop=(fc == FC - 1),
                                )
                            ks = kspool.tile([P, QTW], F32R, tag="ks")
                            nc.scalar.activation(
                                ks[:], ps[:], ACT.Identity,
                                bias=tbk[:, ofc:ofc + 1])
                            nc.sync.dma_start(
                                kt_scr[:, ofc, tt * QTW:(tt + 1) * QTW], ks[:])

                # --- V projection: V[t, of] (natural), no bias (folded) ---
                for half in range(2):  # of-halves: heads 8*half..8*half+7
                    wtile = wpool.tile([P, FC, D // 2], F32R, tag="w_half")
                    nc.sync.dma_start(
                        wtile[:],
                        wv[:, half * 512:(half + 1) * 512].rearrange(
                            "(fc p) o -> p fc o", p=P),
                    )
                    for ktg in range(4):  # groups of 4 key tiles (512 tokens)
                        xts = []
                        for fc in range(FC):
                            xt = xpool.tile([P, QTW], F32R, tag=f"x{fc}")
                            nc.sync.dma_start(
                                xt[:], xv[fc * P:(fc + 1) * P,
                                          ktg * QTW:(ktg + 1) * QTW])
                            xts.append(xt)
                        for ki in range(4):
                            kt = ktg * 4 + ki
                            ps = apsum.tile([P, QTW], F32, tag="acc")
                            for fc in range(FC):
                                nc.tensor.matmul(
                                    ps[:],
                                    xts[fc][:, ki * P:(ki + 1) * P],
                                    wtile[:, fc, :],
                                    start=(fc == 0), stop=(fc == FC - 1),
                                )
                            # scatter 512 of-cols (= 4 pairs x 2 heads x 64)
                            # to DRAM scratch via SBUF staging
                            vs = kspool.tile([P, QTW], F32R, tag="vs")
                            nc.vector.tensor_copy(vs[:], ps[:])
                            nc.sync.dma_start(
                                v_scr[:, half * 4:(half + 1) * 4, kt, :, :],
                                vs[:].rearrange(
                                    "p (c h2 d) -> p c h2 d", c=4, h2=2),
                            )

            # ---------------- Stage B: attention per head ----------------
            with (
                tc.tile_pool(name="kpairpool", bufs=2) as kpp,
                tc.tile_pool(name="ppool", bufs=2) as ppool,
                tc.tile_pool(name="utpool", bufs=2) as utp,
                tc.tile_pool(name="bpsum", bufs=2, space="PSUM") as bpsum,
            ):
                for c in range(PAIRS):
                    kpair = kpp.tile([P, S], F32R, tag="kpair")
                    nc.sync.dma_start(kpair[:], kt_scr[:, c, :])
                    vpair = kpp.tile([P, NKT, 2, DK + 1], F32R, tag="vpair")
                    nc.sync.dma_start(vpair[:, :, :, 0:DK], v_scr[:, c])
                    nc.vector.tensor_copy(vpair[:, :, :, DK:DK + 1], vones_s[:])
                    for h2 in range(2):
                        h = 2 * c + h2
                        base = h2 * DK
                        for qt in range(NQT):
                            qsl = slice(qt * QTW, (qt + 1) * QTW)
                            pt = ppool.tile([P, NKT, QTW], F32R, tag="ptile")
                            for kt in range(NKT):
                                sp = bpsum.tile([P, QTW], F32, tag="scores")
                                nc.tensor.matmul(
                                    sp[:],
                                    kpair[base:base + DK, kt * P:(kt + 1) * P],
                                    qt_s[base:base + DK, c, qsl],
                                    start=True, stop=True,
                                )
                                nc.scalar.activation(
                                    pt[:, kt, :], sp[:], ACT.Exp,
                                    scale=float(SCALE))
                            ut = bpsum.tile([DK + 1, QTW], F32, tag="ut")
                            for kt in range(NKT):
                                nc.tensor.matmul(
                                    ut[:],
                                    vpair[:, kt, h2, :],
                                    pt[:, kt, :],
                                    start=(kt == 0), stop=(kt == NKT - 1),
                                )
                            sums = npool.tile([1, QTW], F32, tag="sums")
                            nc.scalar.activation(sums[:], ut[DK:DK + 1, :],
                                                 ACT.Copy)
                            recip = npool.tile([1, QTW], F32, tag="recip")
                            nc.vector.reciprocal(recip[:], sums[:])
                            bc = bpsum.tile([P, QTW], F32, tag="bc")
                            nc.tensor.matmul(bc[:], tones[:], recip[:],
                                             start=True, stop=True)
                            uts = utp.tile([DK, QTW], F32, tag="uts")
                            nc.scalar.activation(uts[:], ut[0:DK, :], ACT.Copy)
                            nc.vector.tensor_mul(
                                attn_t[base:base + DK, c, qsl],
                                uts[:], bc[0:DK, :])

            # ---------------- Stage C: out projection (transposed) --------
            with (
                tc.tile_pool(name="wopool", bufs=2) as wop,
                tc.tile_pool(name="opool", bufs=3) as opool,
                tc.tile_pool(name="cpsum", bufs=3, space="PSUM") as cpsum,
            ):
                for half in range(2):
                    wtile = wop.tile([P, FC, D // 2], F32R, tag="wo_half")
                    nc.sync.dma_start(
                        wtile[:],
                        wo[:, half * 512:(half + 1) * 512].rearrange(
                            "(fc p) o -> p fc o", p=P),
                    )
                    for tt in range(NQT):
                        for oi in range(4):
                            ofc = half * 4 + oi
                            ps = cpsum.tile([P, QTW], F32, tag="oacc")
                            for c in range(PAIRS):
                                nc.tensor.matmul(
                                    ps[:],
                                    wtile[:, c, oi * P:(oi + 1) * P],
                                    attn_t[:, c, tt * QTW:(tt + 1) * QTW],
                                    start=(c == 0), stop=(c == PAIRS - 1),
                                )
                            osb = opool.tile([P, QTW], F32, tag="osb")
                            nc.scalar.activation(
                                osb[:], ps[:], ACT.Identity,
                                bias=tbo[:, ofc:ofc + 1])
                            nc.sync.dma_start(
                                out[ofc * P:(ofc + 1) * P,
                                    tt * QTW:(tt + 1) * QTW], osb[:])

    nc.finalize()
    return nc


def _prep_host(query, key, value, W_q, b_q, W_k, b_k, W_v, b_v, W_out, b_out):
    """Host-side layout prep (transposes / bias folding). No math beyond the
    b_v fold, which is a 1024x1024 matvec."""
    f32 = np.float32
    query = np.asarray(query, f32)
    key = np.asarray(key, f32)
    value = np.asarray(value, f32)
    W_q = np.asarray(W_q, f32)
    W_k = np.asarray(W_k, f32)
    W_v = np.asarray(W_v, f32)
    W_out = np.asarray(W_out, f32)
    b_q = np.asarray(b_q, f32)
    b_k = np.asarray(b_k, f32)
    b_v = np.asarray(b_v, f32)
    b_out = np.asarray(b_out, f32)

    common = {
        "wq_t": np.ascontiguousarray(W_q.T),
        "wk_t": np.ascontiguousarray(W_k.T),
        "wv_t": np.ascontiguousarray(W_v.T),
        "wo_t": np.ascontiguousarray(W_out.T),
        "b_q_r": np.ascontiguousarray(b_q.reshape(OFC, P).T),
        "b_k_r": np.ascontiguousarray(b_k.reshape(OFC, P).T),
        "b_o_r": np.ascontiguousarray(
            (b_out + W_out @ b_v).reshape(OFC, P).T.astype(f32)),
        "ones_row": np.ones((1, P), f32),
        "v_ones": np.ones((P, NKT, 2, 1), f32),
    }
    in_maps = []
    for c in range(NCORES):
        b, hf = divmod(c, 2)
        m = dict(common)
        m["xq_t"] = np.ascontiguousarray(
            query[b, hf * SQ:(hf + 1) * SQ, :].T)
        m["xk_t"] = np.ascontiguousarray(key[b].T)
        m["xv_t"] = np.ascontiguousarray(value[b].T)
        in_maps.append(m)
    return in_maps


_NC_CACHE = {}


def get_nc():
    if "nc" not in _NC_CACHE:
        _NC_CACHE["nc"] = build_nc()
    return _NC_CACHE["nc"]


def get_runner():
    """Build (once) a cached jitted SPMD callable over 8 cores.

    Mirrors concourse.bass2jax.run_bass_via_pjrt's multi-core path, but keeps
    the jitted function so repeated calls don't recompile the NEFF.
    """
    if "runner" in _NC_CACHE:
        return _NC_CACHE["runner"]

    import jax
    from jax.experimental.shard_map import shard_map
    from jax.sharding import Mesh, PartitionSpec

    from concourse import bass2jax

    nc = get_nc()
    bass2jax.install_neuronx_cc_hook()
    partition_name = (
        nc.partition_id_tensor.name if nc.partition_id_tensor else None
    )

    in_names, out_names, out_avals, zero_shapes = [], [], [], []
    for alloc in nc.m.functions[0].allocations:
        if not isinstance(alloc, mybir.MemoryLocationSet):
            continue
        name = alloc.memorylocations[0].name
        if alloc.kind == "ExternalInput":
            if name != partition_name:
                in_names.append(name)
        elif alloc.kind == "ExternalOutput":
            shape = tuple(alloc.tensor_shape)
            dtype = mybir.dt.np(alloc.dtype)
            out_names.append(name)
            out_avals.append(jax.core.ShapedArray(shape, dtype))
            zero_shapes.append((shape, dtype))
    n_params = len(in_names)
    n_outs = len(out_names)
    all_names = in_names + out_names
    if partition_name is not None:
        all_names = all_names + [partition_name]
    donate = tuple(range(n_params, n_params + n_outs))

    def _body(*args):
        operands = list(args)
        if partition_name is not None:
            operands.append(bass2jax.partition_id_tensor())
        outs = bass2jax._bass_exec_p.bind(
            *operands,
            out_avals=tuple(out_avals),
            in_names=tuple(all_names),
            out_names=tuple(out_names),
            lowering_input_output_aliases=(),
            sim_require_finite=True,
            sim_require_nnan=True,
            nc=nc,
        )
        return tuple(outs)

    devices = jax.devices()[:NCORES]
    mesh = Mesh(np.asarray(devices), ("core",))
    in_specs = (PartitionSpec("core"),) * (n_params + n_outs)
    out_specs = (PartitionSpec("core"),) * n_outs
    sharded = jax.jit(
        shard_map(_body, mesh=mesh, in_specs=in_specs, out_specs=out_specs,
                  check_rep=False),
        donate_argnums=donate,
        keep_unused=True,
    )

    def run(in_maps):
        concat_in = [
            np.concatenate([np.asarray(in_maps[c][n]) for c in range(NCORES)],
                           axis=0)
            for n in in_names
        ]
        zeros = [np.zeros((NCORES * s[0], *s[1:]), d) for s, d in zero_shapes]
        out_arrs = sharded(*concat_in, *zeros)
        return [
            {
                n: np.asarray(out_arrs[i]).reshape(
                    NCORES, *out_avals[i].shape)[c]
                for i, n in enumerate(out_names)
            }
            for c in range(NCORES)
        ]

    runner = {
        "run": run,
        "sharded": sharded,
        "in_names": in_names,
        "out_names": out_names,
        "out_avals": out_avals,
        "zero_shapes": zero_shapes,
        "mesh": mesh,
    }
    _NC_CACHE["runner"] = runner
    return runner


def kernel(**inputs) -> np.ndarray:
    in_maps = _prep_host(**inputs)
    results = get_runner()["run"](in_maps)
    out = np.empty((B, S, D), np.float32)
    for c in range(NCORES):
        b, hf = divmod(c, 2)
        out[b, hf * SQ:(hf + 1) * SQ, :] = results[c]["out_t"].T
    return out


# revision 13
# speedup vs baseline: 1.3356x; 1.3356x over previous
"""Self-contained 8-core Trainium2 Bass kernel for nn_MultiHeadAttention.

Full (unsharded) inputs in, full output out. Sharding: core c handles
batch b = c // 2, query-half h = c % 2 (1024 queries). K/V projections for a
batch are computed redundantly on the 2 cores sharing it -> zero collectives,
disjoint outputs.

All matmuls run in float32r (TF32-like, ~1.6e-4 rel err, 4x fp32 throughput).
"""

import numpy as np

import concourse.bass as bass
import concourse.mybir as mybir
from concourse import bacc
from concourse.tile import TileContext
from concourse.bass_utils import run_bass_kernel_spmd

F32 = mybir.dt.float32
F32R = mybir.dt.float32r
ACT = mybir.ActivationFunctionType

B, S, D = 4, 2048, 1024
H, DK = 16, 64
SQ = S // 2            # queries per core
P = 128
NCORES = 8
FC = D // P            # 8 feature chunks (contraction)
OFC = D // P           # 8 output-feature chunks
PAIRS = H // 2         # 8 head pairs (2 heads = 128 partitions)
NKT = S // P           # 16 key tiles of 128 tokens
QTW = 512              # q tile width
NQT = SQ // QTW        # 2
EB = 2                 # key tiles per exp batch (2 psum banks per scores tile)
SCALE = 1.0 / np.sqrt(np.float32(DK))


def build_nc():
    nc = bacc.Bacc()

    xq = nc.declare_dram_parameter("xq_t", [D, SQ], F32R, isOutput=False)
    xk = nc.declare_dram_parameter("xk_t", [D, S], F32R, isOutput=False)
    xv = nc.declare_dram_parameter("xv_t", [D, S], F32R, isOutput=False)
    wq = nc.declare_dram_parameter("wq_t", [D, D], F32R, isOutput=False)
    wk = nc.declare_dram_parameter("wk_t", [D, D], F32R, isOutput=False)
    wv = nc.declare_dram_parameter("wv_t", [D, D], F32R, isOutput=False)
    wo = nc.declare_dram_parameter("wo_t", [D, D], F32R, isOutput=False)
    bq = nc.declare_dram_parameter("b_q_r", [P, OFC], F32, isOutput=False)
    bk = nc.declare_dram_parameter("b_k_r", [P, OFC], F32, isOutput=False)
    bo = nc.declare_dram_parameter("b_o_r", [P, OFC], F32, isOutput=False)
    ones_in = nc.declare_dram_parameter("ones_row", [1, P], F32, isOutput=False)
    vones_in = nc.declare_dram_parameter("v_ones", [P, NKT, 2, 1], F32R,
                                         isOutput=False)
    out = nc.declare_dram_parameter("out_t", [D, SQ], F32, isOutput=True)

    kt_scr = nc.dram_tensor("kt_scr", [P, PAIRS, S], F32R)
    v_scr = nc.dram_tensor("v_scr", [P, PAIRS, NKT, 2, DK], F32R)

    with nc.allow_low_precision(reason="f32r compute"), TileContext(nc) as tc:
        with (
            tc.tile_pool(name="persist", bufs=1) as pers,
            tc.tile_pool(name="norm", bufs=2) as npool,
        ):
            qt_s = pers.tile([P, OFC, SQ], F32R, tag="qt")
            attn_t = pers.tile([P, PAIRS, SQ], F32R, tag="attnt")
            tbq = pers.tile([P, OFC], F32, tag="tbq")
            tbk = pers.tile([P, OFC], F32, tag="tbk")
            tbo = pers.tile([P, OFC], F32, tag="tbo")
            tones = pers.tile([1, P], F32, tag="tones")
            vones_s = pers.tile([P, NKT, 2, 1], F32R, tag="vones")
            nc.sync.dma_start(tbq[:], bq[:])
            nc.sync.dma_start(tbk[:], bk[:])
            nc.sync.dma_start(tbo[:], bo[:])
            nc.sync.dma_start(tones[:], ones_in[:])
            nc.sync.dma_start(vones_s[:], vones_in[:])

            # ---------------- Stage A: projections ----------------
            with (
                tc.tile_pool(name="wpool", bufs=2) as wpool,
                tc.tile_pool(name="xpool", bufs=2) as xpool,
                tc.tile_pool(name="kspool", bufs=3) as kspool,
                tc.tile_pool(name="apsum", bufs=4, space="PSUM") as apsum,
            ):
                def load_w(src):
                    wt = wpool.tile([P, FC, D], F32R, tag="w_full")
                    for fc in range(FC):
                        nc.sync.dma_start(wt[:, fc, :],
                                          src[fc * P:(fc + 1) * P, :])
                    return wt

                def load_x(src, tt):
                    xts = []
                    for fc in range(FC):
                        xt = xpool.tile([P, QTW], F32R, tag=f"x{fc}")
                        nc.sync.dma_start(
                            xt[:], src[fc * P:(fc + 1) * P,
                                       tt * QTW:(tt + 1) * QTW])
                        xts.append(xt)
                    return xts

                # --- Q projection: QT[of, t] = Wq @ xq ---
                wtq = load_w(wq)
                for tt in range(NQT):
                    xts = load_x(xq, tt)
                    for ofc in range(OFC):
                        ps = apsum.tile([P, QTW], F32, tag="acc")
                        for fc in range(FC):
                            nc.tensor.matmul(
                                ps[:],
                                wtq[:, fc, ofc * P:(ofc + 1) * P],
                                xts[fc][:],
                                start=(fc == 0), stop=(fc == FC - 1),
                            )
                        nc.vector.tensor_scalar_add(
                            qt_s[:, ofc, tt * QTW:(tt + 1) * QTW], ps[:],
                            tbq[:, ofc:ofc + 1])

                # --- K projection -> kt_scr[of, t] (DRAM bounce) ---
                wtk = load_w(wk)
                for tt in range(S // QTW):
                    xts = load_x(xk, tt)
                    for ofc in range(OFC):
                        ps = apsum.tile([P, QTW], F32, tag="acc")
                        for fc in range(FC):
                            nc.tensor.matmul(
                                ps[:],
                                wtk[:, fc, ofc * P:(ofc + 1) * P],
                                xts[fc][:],
                                start=(fc == 0), stop=(fc == FC - 1),
                            )
                        ks = kspool.tile([P, QTW], F32R, tag="ks")
                        nc.vector.tensor_scalar_add(ks[:], ps[:],
                                                    tbk[:, ofc:ofc + 1])
                        nc.sync.dma_start(
                            kt_scr[:, ofc, tt * QTW:(tt + 1) * QTW], ks[:])

                # --- V projection: V[t, of] (natural), no bias (folded) ---
                wtv = load_w(wv)
                for ktg in range(4):  # groups of 4 key tiles (512 tokens)
                    xts = load_x(xv, ktg)
                    for ki in range(4):
                        kt = ktg * 4 + ki
                        for half in range(2):
                            ps = apsum.tile([P, QTW], F32, tag="acc")
                            for fc in range(FC):
                                nc.tensor.matmul(
                                    ps[:],
                                    xts[fc][:, ki * P:(ki + 1) * P],
                                    wtv[:, fc, half * 512:(half + 1) * 512],
                                    start=(fc == 0), stop=(fc == FC - 1),
                                )
                            # scatter 512 of-cols (= 4 pairs x 2 heads x 64)
                            # to DRAM scratch via SBUF staging
                            vs = kspool.tile([P, QTW], F32R, tag="vs")
                            nc.vector.tensor_copy(vs[:], ps[:])
                            nc.sync.dma_start(
                                v_scr[:, half * 4:(half + 1) * 4, kt, :, :],
                                vs[:].rearrange(
                                    "p (c h2 d) -> p c h2 d", c=4, h2=2),
                            )

            # ---------------- Stage B: attention per head ----------------
            with (
                tc.tile_pool(name="kpairpool", bufs=2) as kpp,
                tc.tile_pool(name="ppool", bufs=2) as ppool,
                tc.tile_pool(name="utpool", bufs=2) as utp,
                tc.tile_pool(name="bpsum", bufs=2, space="PSUM") as bpsum,
            ):
                for c in range(PAIRS):
                    kpair = kpp.tile([P, S], F32R, tag="kpair")
                    nc.sync.dma_start(kpair[:], kt_scr[:, c, :])
                    vpair = kpp.tile([P, NKT, 2, DK + 1], F32R, tag="vpair")
                    nc.sync.dma_start(vpair[:, :, :, 0:DK], v_scr[:, c])
                    nc.vector.tensor_copy(vpair[:, :, :, DK:DK + 1], vones_s[:])
                    for h2 in range(2):
                        h = 2 * c + h2
                        base = h2 * DK
                        for qt in range(NQT):
                            qsl = slice(qt * QTW, (qt + 1) * QTW)
                            pt = ppool.tile([P, NKT, QTW], F32R, tag="ptile")
                            for ktb in range(NKT // EB):
                                sp = bpsum.tile([P, EB * QTW], F32,
                                                tag="scores")
                                for e in range(EB):
                                    kt = ktb * EB + e
                                    nc.tensor.matmul(
                                        sp[:, e * QTW:(e + 1) * QTW],
                                        kpair[base:base + DK,
                                              kt * P:(kt + 1) * P],
                                        qt_s[base:base + DK, c, qsl],
                                        start=True, stop=True,
                                    )
                                nc.scalar.activation(
                                    pt[:, ktb * EB:(ktb + 1) * EB, :], sp[:],
                                    ACT.Exp, scale=float(SCALE))
                            ut = bpsum.tile([DK + 1, QTW], F32, tag="ut")
                            for kt in range(NKT):
                                nc.tensor.matmul(
                                    ut[:],
                                    vpair[:, kt, h2, :],
                                    pt[:, kt, :],
                                    start=(kt == 0), stop=(kt == NKT - 1),
                                )
                            recip = npool.tile([1, QTW], F32, tag="recip")
                            nc.vector.reciprocal(recip[:], ut[DK:DK + 1, :])
                            bc = bpsum.tile([P, QTW], F32, tag="bc")
                            nc.tensor.matmul(bc[:], tones[:], recip[:],
                                             start=True, stop=True)
                            uts = utp.tile([DK, QTW], F32, tag="uts")
                            nc.vector.tensor_copy(uts[:], ut[0:DK, :])
                            nc.vector.tensor_mul(
                                attn_t[base:base + DK, c, qsl],
                                uts[:], bc[0:DK, :])

            # ---------------- Stage C: out projection (transposed) --------
            with (
                tc.tile_pool(name="wopool", bufs=2) as wop,
                tc.tile_pool(name="opool", bufs=3) as opool,
                tc.tile_pool(name="cpsum", bufs=3, space="PSUM") as cpsum,
            ):
                for half in range(2):
                    wtile = wop.tile([P, FC, D // 2], F32R, tag="wo_half")
                    for fc in range(FC):
                        nc.sync.dma_start(
                            wtile[:, fc, :],
                            wo[fc * P:(fc + 1) * P,
                               half * 512:(half + 1) * 512])
                    for tt in range(NQT):
                        for oi in range(4):
                            ofc = half * 4 + oi
                            ps = cpsum.tile([P, QTW], F32, tag="oacc")
                            for c in range(PAIRS):
                                nc.tensor.matmul(
                                    ps[:],
                                    wtile[:, c, oi * P:(oi + 1) * P],
                                    attn_t[:, c, tt * QTW:(tt + 1) * QTW],
                                    start=(c == 0), stop=(c == PAIRS - 1),
                                )
                            osb = opool.tile([P, QTW], F32, tag="osb")
                            nc.vector.tensor_scalar_add(osb[:], ps[:],
                                                        tbo[:, ofc:ofc + 1])
                            nc.sync.dma_start(
                                out[ofc * P:(ofc + 1) * P,
                                    tt * QTW:(tt + 1) * QTW], osb[:])

    nc.finalize()
    return nc


def _prep_host(query, key, value, W_q, b_q, W_k, b_k, W_v, b_v, W_out, b_out):
    """Host-side layout prep (transposes / bias folding). No math beyond the
    b_v fold, which is a 1024x1024 matvec."""
    f32 = np.float32
    query = np.asarray(query, f32)
    key = np.asarray(key, f32)
    value = np.asarray(value, f32)
    W_q = np.asarray(W_q, f32)
    W_k = np.asarray(W_k, f32)
    W_v = np.asarray(W_v, f32)
    W_out = np.asarray(W_out, f32)
    b_q = np.asarray(b_q, f32)
    b_k = np.asarray(b_k, f32)
    b_v = np.asarray(b_v, f32)
    b_out = np.asarray(b_out, f32)

    common = {
        "wq_t": np.ascontiguousarray(W_q.T),
        "wk_t": np.ascontiguousarray(W_k.T),
        "wv_t": np.ascontiguousarray(W_v.T),
        "wo_t": np.ascontiguousarray(W_out.T),
        "b_q_r": np.ascontiguousarray(b_q.reshape(OFC, P).T),
        "b_k_r": np.ascontiguousarray(b_k.reshape(OFC, P).T),
        "b_o_r": np.ascontiguousarray(
            (b_out + W_out @ b_v).reshape(OFC, P).T.astype(f32)),
        "ones_row": np.ones((1, P), f32),
        "v_ones": np.ones((P, NKT, 2, 1), f32),
    }
    in_maps = []
    for c in range(NCORES):
        b, hf = divmod(c, 2)
        m = dict(common)
        m["xq_t"] = np.ascontiguousarray(
            query[b, hf * SQ:(hf + 1) * SQ, :].T)
        m["xk_t"] = np.ascontiguousarray(key[b].T)
        m["xv_t"] = np.ascontiguousarray(value[b].T)
        in_maps.append(m)
    return in_maps


_NC_CACHE = {}


def get_nc():
    if "nc" not in _NC_CACHE:
        _NC_CACHE["nc"] = build_nc()
    return _NC_CACHE["nc"]


def get_runner():
    """Build (once) a cached jitted SPMD callable over 8 cores.

    Mirrors concourse.bass2jax.run_bass_via_pjrt's multi-core path, but keeps
    the jitted function so repeated calls don't recompile the NEFF.
    """
    if "runner" in _NC_CACHE:
        return _NC_CACHE["runner"]

    import jax
    from jax.experimental.shard_map import shard_map
    from jax.sharding import Mesh, PartitionSpec

    from concourse import bass2jax

    nc = get_nc()
    bass2jax.install_neuronx_cc_hook()
    partition_name = (
        nc.partition_id_tensor.name if nc.partition_id_tensor else None
    )

    in_names, out_names, out_avals, zero_shapes = [], [], [], []
    for alloc in nc.m.functions[0].allocations:
        if not isinstance(alloc, mybir.MemoryLocationSet):
            continue
        name = alloc.memorylocations[0].name
        if alloc.kind == "ExternalInput":
            if name != partition_name:
                in_names.append(name)
        elif alloc.kind == "ExternalOutput":
            shape = tuple(alloc.tensor_shape)
            dtype = mybir.dt.np(alloc.dtype)
            out_names.append(name)
            out_avals.append(jax.core.ShapedArray(shape, dtype))
            zero_shapes.append((shape, dtype))
    n_params = len(in_names)
    n_outs = len(out_names)
    all_names = in_names + out_names
    if partition_name is not None:
        all_names = all_names + [partition_name]
    donate = tuple(range(n_params, n_params + n_outs))

    def _body(*args):
        operands = list(args)
        if partition_name is not None:
            operands.append(bass2jax.partition_id_tensor())
        outs = bass2jax._bass_exec_p.bind(
            *operands,
            out_avals=tuple(out_avals),
            in_names=tuple(all_names),
            out_names=tuple(out_names),
            lowering_input_output_aliases=(),
            sim_require_finite=True,
            sim_require_nnan=True,
            nc=nc,
        )
        return tuple(outs)

    devices = jax.devices()[:NCORES]
    mesh = Mesh(np.asarray(devices), ("core",))
    in_specs = (PartitionSpec("core"),) * (n_params + n_outs)
    out_specs = (PartitionSpec("core"),) * n_outs
    sharded = jax.jit(
        shard_map(_body, mesh=mesh, in_specs=in_specs, out_specs=out_specs,
                  check_rep=False),
        donate_argnums=donate,
        keep_unused=True,
    )

    def run(in_maps):
        concat_in = [
            np.concatenate([np.asarray(in_maps[c][n]) for c in range(NCORES)],
                           axis=0)
            for n in in_names
        ]
        zeros = [np.zeros((NCORES * s[0], *s[1:]), d) for s, d in zero_shapes]
        out_arrs = sharded(*concat_in, *zeros)
        return [
            {
                n: np.asarray(out_arrs[i]).reshape(
                    NCORES, *out_avals[i].shape)[c]
                for i, n in enumerate(out_names)
            }
            for c in range(NCORES)
        ]

    runner = {
        "run": run,
        "sharded": sharded,
        "in_names": in_names,
        "out_names": out_names,
        "out_avals": out_avals,
        "zero_shapes": zero_shapes,
        "mesh": mesh,
    }
    _NC_CACHE["runner"] = runner
    return runner


def kernel(**inputs) -> np.ndarray:
    in_maps = _prep_host(**inputs)
    results = get_runner()["run"](in_maps)
    out = np.empty((B, S, D), np.float32)
    for c in range(NCORES):
        b, hf = divmod(c, 2)
        out[b, hf * SQ:(hf + 1) * SQ, :] = results[c]["out_t"].T
    return out
